# revision 5
# baseline (speedup 1.0000x reference)
"""Trainium2 Bass kernel for nn_Backbone GNN message-passing layer.

Strategy (8 NeuronCores, SPMD, no collectives):
  - Destination-node-range sharding: core c owns nodes [c*6250, (c+1)*6250)
    and all edges whose dst falls in that range.  Segment softmax and
    segment sum are then core-local.
  - Within a core, edges are grouped into 49 "windows" of 128 dst nodes.
    Segment reductions are PSUM matmuls against a one-hot matrix
    S[e, n] = (rank(e) == n); the per-window accumulator [128, 136] holds
    the weighted-message sum (128) and softmax denominators (8 heads).
    exp max-subtraction is skipped (logits are O(1) by construction).
  - q is never gathered per-edge: within a window all dst nodes come from
    one 128-row block of the q table, so q[e] = S_T^T @ Q_win via matmul
    (S_T[n, e] = (n == rank(e))).
  - Edge-attr LayerNorm without transposes: ea arrives feature-major;
    mean comes from an extra ones-column in the projection matmul, sum of
    squares from a second matmul with squared operand; the rsqrt(var)
    scale is applied on the PSUM->SBUF copy fused with the node-kv add
    (scalar_tensor_tensor).
  - Node features are layer-normed + projected once per core from a
    host-transposed x (feature-major), kvn = rs*(x@Wc_kv)+b staged to
    DRAM in 0.5MB chunks, then fetched per-edge with dma_gather (int16
    indices; table split at row 32768 so indices fit int16).
  - LayerNorm mean-centering is folded into weights (Cn = I - 11^T/128),
    so on-chip LN is just the rsqrt(var) scale.
  - FFN (+ residuals) runs per window right out of PSUM.

Host-side preprocessing is index/layout work: bucketing edges by
(core, window, src-half), padding to uniform capacity so one SPMD program
serves all cores, permuting/transposing edge_attr and x, folding LN
affine constants into weights.
"""

import os
import numpy as np
import ml_dtypes
from contextlib import ExitStack

import concourse.bacc as bacc
import concourse.bass as bass
import concourse.tile as tile
import concourse.mybir as mybir
from concourse.bass_utils import run_bass_kernel_spmd

bf16 = ml_dtypes.bfloat16
F32 = mybir.dt.float32
BF = mybir.dt.bfloat16
I16 = mybir.dt.int16

N, E, H, NH, HD = 50000, 800000, 128, 8, 16
NCORES = 8
NPC = N // NCORES            # 6250 nodes per core
P = 128
NW = -(-NPC // P)            # 49 windows per core
EPS = 1e-5
MACRO = 4                    # subtiles per macro-tile
MACRO_N = 8                  # node-phase tiles per staging group
SPLIT = 32768                # node-table split so gather indices fit int16
NODE_PAD = 50176             # 392 * 128
QROWS = NW * P               # 6272 padded own-range rows
GC = 1024                    # max indices per dma_gather call

AF = mybir.ActivationFunctionType
ALU = mybir.AluOpType


def _ceil(a, b):
    return -(-a // b)


def _wrap16(a):
    """[..., L] int16 -> [..., 128, L//16] gather-index layout
    (idx i at partition i%16, col i//16; replicated 8x across partitions)."""
    sh = a.shape[:-1]
    L = a.shape[-1]
    w = a.reshape(*sh, L // 16, 16)
    w = np.swapaxes(w, -1, -2)  # [..., 16, L//16]
    reps = (1,) * len(sh) + (8, 1)
    return np.ascontiguousarray(np.tile(w, reps))


def _prep(inputs):
    x = np.asarray(inputs["x"], np.float32)
    ei = np.asarray(inputs["edge_index"])
    ea = np.asarray(inputs["edge_attr"], np.float32)
    f32 = np.float32
    Wq, Wk, Wv = (np.asarray(inputs[k], f32) for k in ("Wq", "Wk", "Wv"))
    Wek, Wev = (np.asarray(inputs[k], f32) for k in ("Wek", "Wev"))
    W1, W2 = np.asarray(inputs["W1"], f32), np.asarray(inputs["W2"], f32)
    bq, bk, bv = (np.asarray(inputs[k], f32) for k in ("bq", "bk", "bv"))
    bek, bev = (np.asarray(inputs[k], f32) for k in ("bek", "bev"))
    b1, b2 = np.asarray(inputs["b1"], f32), np.asarray(inputs["b2"], f32)
    lsw, lsb = np.asarray(inputs["ln_src_w"], f32), np.asarray(inputs["ln_src_b"], f32)
    lew, leb = np.asarray(inputs["ln_edge_w"], f32), np.asarray(inputs["ln_edge_b"], f32)
    lfw, lfb = np.asarray(inputs["ln_ffn_w"], f32), np.asarray(inputs["ln_ffn_b"], f32)

    src = ei[0].astype(np.int64)
    dst = ei[1].astype(np.int64)

    core = dst // NPC
    dstl = dst - core * NPC
    win = dstl >> 7
    rank = dstl & 127
    half = (src >= SPLIT).astype(np.int64)
    group = (core * NW + win) * 2 + half
    NG = NCORES * NW * 2
    counts = np.bincount(group, minlength=NG)

    A_sub = 4 * max(1, _ceil(int(counts[0::2].max()), 4 * P))
    B_sub = 4 * max(1, _ceil(int(counts[1::2].max()), 4 * P))
    W_SUB = A_sub + B_sub
    AE, WE = A_sub * P, W_SUB * P
    E_pad = NW * WE
    S_total = E_pad // P

    # target slot for each edge in the padded per-core layout
    order = np.argsort(group, kind="stable")
    gs = group[order]
    starts = np.zeros(NG + 1, np.int64)
    np.cumsum(counts, out=starts[1:])
    within = np.arange(E, dtype=np.int64) - starts[gs]
    tgt = (gs // (2 * NW)) * E_pad + ((gs // 2) % NW) * WE + (gs & 1) * AE + within

    eid = np.full(NCORES * E_pad, -1, np.int64)
    eid[tgt] = order
    valid = eid >= 0
    eiv = eid[valid]

    ea_pad = np.zeros((NCORES * E_pad, H), bf16)
    ea_pad[valid] = ea.astype(bf16)[eiv]
    eaT = np.ascontiguousarray(
        ea_pad.reshape(NCORES, E_pad, H).transpose(0, 2, 1)
    )  # [8, 128, E_pad]

    kvidx = np.zeros(NCORES * E_pad, np.int64)  # pads gather row 0 (harmless)
    kvidx[valid] = src[eiv] - SPLIT * half[eiv]
    kvidx = kvidx.astype(np.int16).reshape(NCORES, NW, WE)
    kvA = _wrap16(kvidx[:, :, :AE])   # [8, NW, 128, AE//16]
    kvB = _wrap16(kvidx[:, :, AE:])   # [8, NW, 128, BE//16]
    kvA = np.ascontiguousarray(kvA.transpose(0, 2, 1, 3))  # [8, 128, NW, AE//16]
    kvB = np.ascontiguousarray(kvB.transpose(0, 2, 1, 3))

    rk = np.full(NCORES * E_pad, 300.0, np.float32)
    rk[valid] = rank[eiv]
    rankpt = np.ascontiguousarray(
        rk.reshape(NCORES, S_total, P).transpose(0, 2, 1)
    ).astype(bf16)  # [8, 128, S_total] bf16: [p, s] = rank of edge s*128+p
    rank_rows = np.ascontiguousarray(
        rk.reshape(NCORES, NW, WE)
    ).astype(bf16)  # [8, NW, WE]: row layout for partition_broadcast

    # feature-major node features
    x_bf = np.zeros((NODE_PAD, H), bf16)
    x_bf[:N] = x.astype(bf16)
    xT_bf = np.ascontiguousarray(x_bf.T)          # [128, NODE_PAD]
    x_ownT = np.zeros((NCORES, H, QROWS), bf16)   # feature-major own range
    x_own_f = np.zeros((NCORES, QROWS, H), np.float32)
    for c in range(NCORES):
        x_ownT[c, :, :NPC] = x_bf[c * NPC:(c + 1) * NPC].T
        x_own_f[c, :NPC] = x[c * NPC:(c + 1) * NPC]

    # LN folding: LN(v) @ W + b  ==  rsqrt(var) * (v @ Wc) + bc, with
    # Wc = (I - 11^T/128) diag(ln_w) W  and  bc = ln_b @ W + b.
    Cn = np.eye(H, dtype=f32) - np.full((H, H), 1.0 / H, f32)
    Wc_k = Cn @ (lsw[:, None] * Wk)
    Wc_v = Cn @ (lsw[:, None] * Wv)
    Wc_q = Cn @ (lsw[:, None] * Wq)
    Wc_ek = Cn @ (lew[:, None] * Wek)
    Wc_ev = Cn @ (lew[:, None] * Wev)
    mean_col = np.full((H, 1), 1.0 / H, f32)
    # [Wc_k | Wc_v | mean] -> [128, 257]
    Wc_kvo = np.concatenate([Wc_k, Wc_v, mean_col], 1).astype(bf16)
    Wc_ekvo = np.concatenate([Wc_ek, Wc_ev, mean_col], 1).astype(bf16)
    Wc_qo = np.concatenate([Wc_q, mean_col], 1).astype(bf16)      # [128, 129]
    ssq_col = mean_col.astype(bf16)                               # [128, 1]
    b_k = lsb @ Wk + bk + leb @ Wek + bek
    b_v = lsb @ Wv + bv + leb @ Wev + bev
    b_kv_rep = np.tile(np.concatenate([b_k, b_v])[None, :], (P, 1)).astype(bf16)
    b_q_rep = np.tile((lsb @ Wq + bq)[None, :], (P, 1)).astype(bf16)
    W1c = (Cn @ (lfw[:, None] * W1)).astype(bf16)             # [128, 512]
    b1_row = (lfb @ W1 + b1)[None, :].astype(bf16)            # [1, 512]
    W2p = np.ascontiguousarray(
        W2.reshape(4, P, H).transpose(1, 0, 2)
    ).astype(bf16)                                            # [128, 4, 128]
    b2_row = b2[None, :].astype(bf16)
    C_iota = np.tile(np.arange(P, dtype=f32)[None, :], (P, 1)).astype(bf16)
    p_iota = np.arange(P, dtype=f32)[:, None]                 # [128, 1] f32
    ident = np.eye(P, dtype=f32).astype(bf16)
    ones_row = np.ones((1, P), bf16)

    shared = dict(
        xT_bf=xT_bf, Wc_kvo=Wc_kvo, Wc_ekvo=Wc_ekvo, Wc_qo=Wc_qo,
        ssq_col=ssq_col, b_kv_rep=b_kv_rep, b_q_rep=b_q_rep,
        W1c=W1c, b1_row=b1_row, W2p=W2p, b2_row=b2_row,
        C_iota=C_iota, p_iota=p_iota, ident=ident, ones_row=ones_row,
    )
    in_maps = []
    for c in range(NCORES):
        m = dict(shared)
        m.update(
            eaT=eaT[c], kvA=kvA[c], kvB=kvB[c],
            rankpt=rankpt[c], rank_rows=rank_rows[c],
            x_ownT=x_ownT[c], x_own_f=x_own_f[c],
        )
        in_maps.append(m)

    cfg = dict(A_sub=A_sub, B_sub=B_sub, W_SUB=W_SUB, E_pad=E_pad,
               S_total=S_total)
    return cfg, in_maps


def _build(cfg):
    A_sub, B_sub = cfg["A_sub"], cfg["B_sub"]
    W_SUB, E_pad = cfg["W_SUB"], cfg["E_pad"]
    AE, BE, WE = A_sub * P, B_sub * P, W_SUB * P
    S_total = cfg["S_total"]
    NMACRO = W_SUB // MACRO

    nc = bacc.Bacc("TRN2", target_bir_lowering=False, debug=False)

    # ---- I/O ----
    xT_bf_d = nc.dram_tensor("xT_bf", [P, NODE_PAD], BF, kind="ExternalInput")
    x_ownT_d = nc.dram_tensor("x_ownT", [P, QROWS], BF, kind="ExternalInput")
    x_own_f_d = nc.dram_tensor("x_own_f", [QROWS, H], F32, kind="ExternalInput")
    eaT_d = nc.dram_tensor("eaT", [P, E_pad], BF, kind="ExternalInput")
    kvA_d = nc.dram_tensor("kvA", [P, NW, AE // 16], I16, kind="ExternalInput")
    kvB_d = nc.dram_tensor("kvB", [P, NW, BE // 16], I16, kind="ExternalInput")
    rankpt_d = nc.dram_tensor("rankpt", [P, S_total], BF, kind="ExternalInput")
    rank_rows_d = nc.dram_tensor("rank_rows", [NW, WE], BF, kind="ExternalInput")
    Wc_kvo_d = nc.dram_tensor("Wc_kvo", [P, 257], BF, kind="ExternalInput")
    Wc_ekvo_d = nc.dram_tensor("Wc_ekvo", [P, 257], BF, kind="ExternalInput")
    Wc_qo_d = nc.dram_tensor("Wc_qo", [P, 129], BF, kind="ExternalInput")
    ssq_col_d = nc.dram_tensor("ssq_col", [P, 1], BF, kind="ExternalInput")
    b_kv_d = nc.dram_tensor("b_kv_rep", [P, 256], BF, kind="ExternalInput")
    b_q_d = nc.dram_tensor("b_q_rep", [P, P], BF, kind="ExternalInput")
    W1c_d = nc.dram_tensor("W1c", [P, 4 * H], BF, kind="ExternalInput")
    b1_d = nc.dram_tensor("b1_row", [1, 4 * H], BF, kind="ExternalInput")
    W2p_d = nc.dram_tensor("W2p", [P, 4, H], BF, kind="ExternalInput")
    b2_d = nc.dram_tensor("b2_row", [1, H], BF, kind="ExternalInput")
    iota_d = nc.dram_tensor("C_iota", [P, P], BF, kind="ExternalInput")
    p_iota_d = nc.dram_tensor("p_iota", [P, 1], F32, kind="ExternalInput")
    ident_d = nc.dram_tensor("ident", [P, P], BF, kind="ExternalInput")
    ones_d = nc.dram_tensor("ones_row", [1, P], BF, kind="ExternalInput")
    out_d = nc.dram_tensor("out", [QROWS, H], F32, kind="ExternalOutput")

    with tile.TileContext(nc) as tc, ExitStack() as ctx:
        const = ctx.enter_context(tc.tile_pool(name="const", bufs=1))

        kvn_t = nc.dram_tensor("kvn_s", [NODE_PAD, 256], BF,
                               kind="ExternalOutput")
        qn_t = nc.dram_tensor("qn_s", [QROWS, H], BF, kind="ExternalOutput")

        # resident constants
        wckvo = const.tile([P, 257], BF)
        wcekvo = const.tile([P, 257], BF)
        wcqo = const.tile([P, 129], BF)
        ssqc = const.tile([P, 1], BF)
        bkv = const.tile([P, 256], BF)
        bqr = const.tile([P, P], BF)
        w1c = const.tile([P, 4 * H], BF)
        b1r = const.tile([1, 4 * H], BF)
        w2p = const.tile([P, 4, H], BF)
        b2r = const.tile([1, H], BF)
        iota = const.tile([P, P], BF)
        piota = const.tile([P, 1], F32)
        idn = const.tile([P, P], BF)
        onesr = const.tile([1, P], BF)
        rank_sb = const.tile([P, S_total], BF)
        rank_rw = const.tile([NW, WE], BF)
        kvA_sb = const.tile([P, NW, AE // 16], I16)
        kvB_sb = const.tile([P, NW, BE // 16], I16)
        eps_c = const.tile([P, 1], F32)
        tiny_c = const.tile([P, 1], F32)
        nc.vector.memset(eps_c[:], EPS)
        nc.vector.memset(tiny_c[:], 1e-16)
        for t, d in ((wckvo, Wc_kvo_d), (wcekvo, Wc_ekvo_d), (wcqo, Wc_qo_d),
                     (ssqc, ssq_col_d), (bkv, b_kv_d), (bqr, b_q_d),
                     (w1c, W1c_d), (b1r, b1_d), (w2p, W2p_d), (b2r, b2_d),
                     (iota, iota_d), (piota, p_iota_d), (idn, ident_d),
                     (onesr, ones_d), (rank_sb, rankpt_d),
                     (rank_rw, rank_rows_d), (kvA_sb, kvA_d), (kvB_sb, kvB_d)):
            nc.sync.dma_start(out=t[:], in_=d[:])

        # ---------------- node phase ----------------
        # kvn = rs * (x @ Wc_kv) + b, processed from feature-major xT.
        # mean comes from col 256 of the proj matmul, ssq from a second
        # matmul with squared operand; rs = rsqrt(E[x^2] - mu^2 + eps).
        def project_nodes(xT_dram, nrows, wc, wid, brep, dst_dram, tag):
            nsub = nrows // P
            with ExitStack() as c2:
                sb = c2.enter_context(tc.tile_pool(name=f"np_{tag}", bufs=3))
                ps = c2.enter_context(
                    tc.tile_pool(name=f"npp_{tag}", bufs=4, space="PSUM"))
                for g in range(0, nsub, MACRO_N):
                    gn = min(MACRO_N, nsub - g)
                    slab = sb.tile([P, MACRO_N, P], BF, tag="slab")
                    nc.sync.dma_start(
                        out=slab[:, 0:gn, :],
                        in_=xT_dram[:, g * P:(g + gn) * P].rearrange(
                            "p (t c) -> p t c", c=P))
                    sq = sb.tile([P, MACRO_N, P], BF, tag="sq")
                    nc.vector.tensor_mul(out=sq[:, 0:gn, :],
                                         in0=slab[:, 0:gn, :],
                                         in1=slab[:, 0:gn, :])
                    stage = sb.tile([P, MACRO_N, wid], BF, tag="stage")
                    stats = sb.tile([P, MACRO_N, 2], F32, tag="stats")
                    for j in range(gn):
                        pp = ps.tile([P, wid + 2], F32, tag="pp")
                        nc.tensor.matmul(out=pp[:, 0:wid + 1],
                                         lhsT=slab[:, j, :], rhs=wc[:],
                                         start=True, stop=True)
                        nc.tensor.matmul(out=pp[:, wid + 1:wid + 2],
                                         lhsT=sq[:, j, :], rhs=ssqc[:],
                                         start=True, stop=True,
                                         skip_group_check=True)
                        # var = E[x^2] - mu^2 ; rs = rsqrt(var + eps)
                        nc.vector.tensor_mul(out=stats[:, j, 0:1],
                                             in0=pp[:, wid:wid + 1],
                                             in1=pp[:, wid:wid + 1])
                        nc.vector.tensor_sub(out=stats[:, j, 1:2],
                                             in0=pp[:, wid + 1:wid + 2],
                                             in1=stats[:, j, 0:1])
                        rs = sb.tile([P, 1], F32, tag="rs")
                        nc.scalar.activation(out=rs[:], in_=stats[:, j, 1:2],
                                             func=AF.Rsqrt, bias=eps_c[:])
                        nc.vector.scalar_tensor_tensor(
                            out=stage[:, j, :], in0=pp[:, 0:wid],
                            scalar=rs[:], in1=brep[:, 0:wid],
                            op0=ALU.mult, op1=ALU.add)
                    rows = gn * P
                    nc.sync.dma_start(
                        out=dst_dram[g * P:g * P + rows, :].rearrange(
                            "(t p) c -> p t c", p=P),
                        in_=stage[:, 0:gn, :])

        project_nodes(xT_bf_d, NODE_PAD, wckvo, 256, bkv, kvn_t, "kv")
        project_nodes(x_ownT_d, QROWS, wcqo, 128, bqr, qn_t, "q")

        # ---------------- edge phase ----------------
        with ExitStack() as c2:
            sbw = c2.enter_context(tc.tile_pool(name="win", bufs=2))
            sbe = c2.enter_context(tc.tile_pool(name="edge", bufs=3))
            ps_pp = c2.enter_context(
                tc.tile_pool(name="ppp", bufs=2, space="PSUM"))
            ps_acc = c2.enter_context(
                tc.tile_pool(name="pacc", bufs=2, space="PSUM"))
            sbf = c2.enter_context(tc.tile_pool(name="ffn", bufs=2))

            for w in range(NW):
                # window-level loads
                ea_slab = sbw.tile([P, W_SUB, P], BF, tag="eas")
                nc.sync.dma_start(out=ea_slab[:],
                                  in_=eaT_d[:, w * WE:(w + 1) * WE].rearrange(
                                      "p (t c) -> p t c", c=P))
                q_win = sbw.tile([P, P], BF, tag="qw")
                nc.sync.dma_start(out=q_win[:],
                                  in_=qn_t[w * P:(w + 1) * P, :])
                rb = sbw.tile([P, W_SUB, P], BF, tag="rb")
                nc.gpsimd.partition_broadcast(rb[:], rank_rw[w:w + 1, :])
                kv_g = sbw.tile([P, W_SUB, 256], BF, tag="kvg")
                for j in range(_ceil(AE, GC)):
                    n_i = min(GC, AE - j * GC)
                    nc.gpsimd.dma_gather(
                        kv_g[:, j * (GC // P):j * (GC // P) + n_i // P, :],
                        kvn_t[0:SPLIT, :],
                        kvA_sb[:, w, j * (GC // 16):j * (GC // 16) + n_i // 16],
                        n_i, n_i, 256)
                for j in range(_ceil(BE, GC)):
                    n_i = min(GC, BE - j * GC)
                    nc.gpsimd.dma_gather(
                        kv_g[:, A_sub + j * (GC // P):
                             A_sub + j * (GC // P) + n_i // P, :],
                        kvn_t[SPLIT:NODE_PAD, :],
                        kvB_sb[:, w, j * (GC // 16):j * (GC // 16) + n_i // 16],
                        n_i, n_i, 256)

                agg = ps_acc.tile([P, 136], F32, tag="acc")

                for m in range(NMACRO):
                    gs0 = w * W_SUB + m * MACRO
                    ea4 = ea_slab[:, m * MACRO:(m + 1) * MACRO, :]
                    sq4 = sbe.tile([P, MACRO, P], BF, tag="sq4")
                    nc.vector.tensor_mul(out=sq4[:], in0=ea4, in1=ea4)
                    pp4 = ps_pp.tile([P, MACRO, 258], F32, tag="pp4")
                    for s in range(MACRO):
                        nc.tensor.matmul(out=pp4[:, s, 0:257],
                                         lhsT=ea4[:, s, :], rhs=wcekvo[:],
                                         start=True, stop=True,
                                         skip_group_check=True)
                        nc.tensor.matmul(out=pp4[:, s, 257:258],
                                         lhsT=sq4[:, s, :], rhs=ssqc[:],
                                         start=True, stop=True,
                                         skip_group_check=True)
                    # rs4 = rsqrt(E[x^2] - mu^2 + eps)
                    st4 = sbe.tile([P, MACRO, 2], F32, tag="st4")
                    nc.vector.tensor_mul(out=st4[:, :, 0:1],
                                         in0=pp4[:, :, 256:257],
                                         in1=pp4[:, :, 256:257])
                    nc.vector.tensor_sub(out=st4[:, :, 1:2],
                                         in0=pp4[:, :, 257:258],
                                         in1=st4[:, :, 0:1])
                    rs4 = sbe.tile([P, MACRO], F32, tag="rs4")
                    nc.scalar.activation(out=rs4[:], in_=st4[:, :, 1],
                                         func=AF.Rsqrt, bias=eps_c[:])
                    # kvf = kv_g + rs * ekv   (fused PSUM->SBUF)
                    kvf4 = sbe.tile([P, MACRO, 256], BF, tag="kvf4")
                    for s in range(MACRO):
                        nc.vector.scalar_tensor_tensor(
                            out=kvf4[:, s, :], in0=pp4[:, s, 0:256],
                            scalar=rs4[:, s:s + 1],
                            in1=kv_g[:, m * MACRO + s, :],
                            op0=ALU.mult, op1=ALU.add)
                    # S_T[n, e] = (n == rank(e)) ; q = S_T^T @ Q_win
                    st_4 = sbe.tile([P, MACRO, P], BF, tag="stq4")
                    nc.vector.tensor_scalar(
                        out=st_4[:], in0=rb[:, m * MACRO:(m + 1) * MACRO, :],
                        scalar1=piota[:], scalar2=None, op0=ALU.is_equal)
                    qt4 = ps_acc.tile([P, MACRO, P], F32, tag="acc")
                    for s in range(MACRO):
                        nc.tensor.matmul(out=qt4[:, s, :],
                                         lhsT=st_4[:, s, :], rhs=q_win[:],
                                         start=True, stop=True,
                                         skip_group_check=True)
                    # logits and softmax numerators
                    qk4 = sbe.tile([P, MACRO, P], BF, tag="qk4")
                    nc.vector.tensor_mul(out=qk4[:], in0=qt4[:],
                                         in1=kvf4[:, :, 0:P])
                    l4 = sbe.tile([P, MACRO, NH], F32, tag="l4")
                    nc.vector.tensor_reduce(
                        out=l4[:],
                        in_=qk4[:].rearrange("p m (h d) -> p m h d", d=HD),
                        axis=mybir.AxisListType.X, op=ALU.add)
                    U4 = sbe.tile([P, MACRO, 136], BF, tag="U4")
                    nc.scalar.activation(out=U4[:, :, P:136], in_=l4[:],
                                         func=AF.Exp, scale=0.25)
                    nc.vector.tensor_mul(
                        out=U4[:, :, 0:P].rearrange("p m (h d) -> p m h d", d=HD),
                        in0=kvf4[:, :, P:256].rearrange("p m (h d) -> p m h d", d=HD),
                        in1=U4[:, :, P:136].unsqueeze(3).broadcast_to(
                            [P, MACRO, NH, HD]))
                    # S[e, n] = (rank(e) == n) ; agg += S^T @ U
                    s4 = sbe.tile([P, MACRO, P], BF, tag="s4")
                    nc.vector.tensor_tensor(
                        out=s4[:],
                        in0=iota[:].unsqueeze(1).broadcast_to([P, MACRO, P]),
                        in1=rank_sb[:, gs0:gs0 + MACRO].unsqueeze(2)
                            .broadcast_to([P, MACRO, P]),
                        op=ALU.is_equal)
                    for s in range(MACRO):
                        nc.tensor.matmul(out=agg[:], lhsT=s4[:, s, :],
                                         rhs=U4[:, s, :],
                                         start=(m == 0 and s == 0),
                                         stop=(m == NMACRO - 1 and s == MACRO - 1))

                # ---- finalize + FFN for this window ----
                rden = sbf.tile([P, NH], F32, tag="rden")
                nc.scalar.activation(out=rden[:], in_=agg[:, P:136],
                                     func=AF.Reciprocal, bias=tiny_c[:])
                xw = sbf.tile([P, H], F32, tag="xw")
                nc.sync.dma_start(out=xw[:],
                                  in_=x_own_f_d[w * P:(w + 1) * P, :])
                aggn = sbf.tile([P, H], F32, tag="aggn")
                nc.vector.tensor_mul(
                    out=aggn[:].rearrange("p (h d) -> p h d", d=HD),
                    in0=agg[:, 0:H].rearrange("p (h d) -> p h d", d=HD),
                    in1=rden[:].unsqueeze(2).broadcast_to([P, NH, HD]))
                xd = sbf.tile([P, H], F32, tag="xd")
                nc.vector.tensor_add(out=xd[:], in0=xw[:], in1=aggn[:])

                st6f = sbf.tile([P, 6], F32, tag="st6f")
                mvf = sbf.tile([P, 2], F32, tag="mvf")
                nc.vector.bn_stats(out=st6f[:], in_=xd[:])
                nc.vector.bn_aggr(out=mvf[:], in_=st6f[:])
                rsf = sbf.tile([P, 1], F32, tag="rsf")
                nc.scalar.activation(out=rsf[:], in_=mvf[:, 1:2],
                                     func=AF.Rsqrt, bias=eps_c[:])
                hp = sbf.tile([P, H], BF, tag="hp")
                nc.vector.tensor_scalar_mul(out=hp[:], in0=xd[:], scalar1=rsf[:])
                hT_ps = ps_acc.tile([P, P], BF, tag="acc")
                nc.tensor.transpose(out=hT_ps[:], in_=hp[:], identity=idn[:])
                hT = sbf.tile([P, P], BF, tag="hT")
                nc.vector.tensor_copy(out=hT[:], in_=hT_ps[:])
                h1 = ps_acc.tile([P, 4 * H], F32, tag="acc")
                nc.tensor.matmul(out=h1[:], lhsT=hT[:], rhs=w1c[:],
                                 start=True, stop=False)
                nc.tensor.matmul(out=h1[:], lhsT=onesr[:], rhs=b1r[:],
                                 start=False, stop=True)
                r = sbf.tile([P, 4 * H], BF, tag="r")
                nc.scalar.activation(out=r[:], in_=h1[:], func=AF.Relu)
                rT_ps = ps_acc.tile([P, 4 * H], BF, tag="acc")
                for k in range(4):
                    nc.tensor.transpose(out=rT_ps[:, k * P:(k + 1) * P],
                                        in_=r[:, k * P:(k + 1) * P],
                                        identity=idn[:])
                rT = sbf.tile([P, 4 * H], BF, tag="rT")
                nc.vector.tensor_copy(out=rT[:], in_=rT_ps[:])
                op = ps_acc.tile([P, H], F32, tag="acc")
                for k in range(4):
                    nc.tensor.matmul(out=op[:], lhsT=rT[:, k * P:(k + 1) * P],
                                     rhs=w2p[:, k, :], start=(k == 0),
                                     stop=False)
                nc.tensor.matmul(out=op[:], lhsT=onesr[:], rhs=b2r[:],
                                 start=False, stop=True)
                ob = sbf.tile([P, H], F32, tag="ob")
                nc.vector.tensor_add(out=ob[:], in0=xd[:], in1=op[:])
                nc.sync.dma_start(out=out_d[w * P:(w + 1) * P, :], in_=ob[:])

    nc.compile()
    return nc


_CACHE = {}


def _get_program(cfg):
    key = tuple(sorted(cfg.items()))
    if key not in _CACHE:
        _CACHE[key] = _build(cfg)
    return _CACHE[key]


def kernel(_collect_results=None, **inputs):
    cfg, in_maps = _prep(inputs)
    nc = _get_program(cfg)
    res = run_bass_kernel_spmd(
        nc, in_maps, core_ids=list(range(NCORES)),
        trace=bool(os.environ.get("GNN_TRACE", "")))
    if _collect_results is not None:
        _collect_results.append(res)
    out = np.empty((N, H), np.float32)
    for c in range(NCORES):
        out[c * NPC:(c + 1) * NPC] = res.results[c]["out"][:NPC]
    return out


# revision 23
# speedup vs baseline: 2.6994x; 2.6994x over previous
"""Trainium2 Bass kernel for nn_Backbone GNN message-passing layer.

Strategy (8 NeuronCores, SPMD, no collectives):
  - Destination-node-range sharding: core c owns nodes [c*6250, (c+1)*6250)
    and all edges whose dst falls in that range.  Segment softmax and
    segment sum are then core-local.
  - Within a core, edges are grouped into 49 "windows" of 128 dst nodes.
    Segment reductions are PSUM matmuls against a one-hot matrix
    S[e, n] = (rank(e) == n); the per-window accumulator [128, 136] holds
    the weighted-message sum (128) and softmax denominators (8 heads).
    exp max-subtraction is skipped (logits are O(1) by construction).
  - q is never gathered per-edge: within a window all dst nodes come from
    one 128-row block of the q table, so q[e] = S_T^T @ Q_win via matmul
    (S_T[n, e] = (n == rank(e))).
  - Edge-attr LayerNorm without transposes: ea arrives feature-major;
    mean comes from an extra ones-column in the projection matmul, sum of
    squares from a second matmul with squared operand; the rsqrt(var)
    scale is applied on the PSUM->SBUF copy fused with the node-kv add
    (scalar_tensor_tensor).
  - Node features are layer-normed + projected once per core from a
    host-transposed x (feature-major), kvn = rs*(x@Wc_kv)+b staged to
    DRAM in 0.5MB chunks, then fetched per-edge with dma_gather (int16
    indices; table split at row 32768 so indices fit int16).
  - LayerNorm mean-centering is folded into weights (Cn = I - 11^T/128),
    so on-chip LN is just the rsqrt(var) scale.
  - FFN (+ residuals) runs per window right out of PSUM.

Host-side preprocessing is index/layout work: bucketing edges by
(core, window, src-half), padding to uniform capacity so one SPMD program
serves all cores, permuting/transposing edge_attr and x, folding LN
affine constants into weights.
"""

import os
import numpy as np
import ml_dtypes
from contextlib import ExitStack

import concourse.bacc as bacc
import concourse.bass as bass
import concourse.tile as tile
import concourse.mybir as mybir
from concourse.bass_utils import run_bass_kernel_spmd

bf16 = ml_dtypes.bfloat16
F32 = mybir.dt.float32
BF = mybir.dt.bfloat16
I16 = mybir.dt.int16

N, E, H, NH, HD = 50000, 800000, 128, 8, 16
NCORES = 8
NPC = N // NCORES            # 6250 nodes per core
P = 128
NW = -(-NPC // P)            # 49 windows per core
EPS = 1e-5
MACRO = 4                    # subtiles per macro-tile
MACRO_N = 8                  # node-phase tiles per staging group
SPLIT = 32768                # node-table split so gather indices fit int16
NODE_PAD = 50176             # 392 * 128
QROWS = NW * P               # 6272 padded own-range rows
GC = 1024                    # max indices per dma_gather call

AF = mybir.ActivationFunctionType
ALU = mybir.AluOpType


def _ceil(a, b):
    return -(-a // b)


def _wrap16(a):
    """[..., L] int16 -> [..., 128, L//16] gather-index layout
    (idx i at partition i%16, col i//16; replicated 8x across partitions)."""
    sh = a.shape[:-1]
    L = a.shape[-1]
    w = a.reshape(*sh, L // 16, 16)
    w = np.swapaxes(w, -1, -2)  # [..., 16, L//16]
    reps = (1,) * len(sh) + (8, 1)
    return np.ascontiguousarray(np.tile(w, reps))


def _prep(inputs):
    x = np.asarray(inputs["x"], np.float32)
    ei = np.asarray(inputs["edge_index"])
    ea = np.asarray(inputs["edge_attr"], np.float32)
    f32 = np.float32
    Wq, Wk, Wv = (np.asarray(inputs[k], f32) for k in ("Wq", "Wk", "Wv"))
    Wek, Wev = (np.asarray(inputs[k], f32) for k in ("Wek", "Wev"))
    W1, W2 = np.asarray(inputs["W1"], f32), np.asarray(inputs["W2"], f32)
    bq, bk, bv = (np.asarray(inputs[k], f32) for k in ("bq", "bk", "bv"))
    bek, bev = (np.asarray(inputs[k], f32) for k in ("bek", "bev"))
    b1, b2 = np.asarray(inputs["b1"], f32), np.asarray(inputs["b2"], f32)
    lsw, lsb = np.asarray(inputs["ln_src_w"], f32), np.asarray(inputs["ln_src_b"], f32)
    lew, leb = np.asarray(inputs["ln_edge_w"], f32), np.asarray(inputs["ln_edge_b"], f32)
    lfw, lfb = np.asarray(inputs["ln_ffn_w"], f32), np.asarray(inputs["ln_ffn_b"], f32)

    src = ei[0].astype(np.int64)
    dst = ei[1].astype(np.int64)

    core = dst // NPC
    dstl = dst - core * NPC
    win = dstl >> 7
    rank = dstl & 127
    half = (src >= SPLIT).astype(np.int64)
    group = (core * NW + win) * 2 + half
    NG = NCORES * NW * 2
    counts = np.bincount(group, minlength=NG)

    # Load-balance: sort each core's windows by edge count descending, so
    # slot j holds each core's j-th fullest window; slot capacity is the max
    # over cores.  One SPMD program with per-slot trip counts serves all
    # cores with ~14% less padding than a global max.
    cntA = counts[0::2].reshape(NCORES, NW)
    cntB = counts[1::2].reshape(NCORES, NW)
    perm = np.argsort(-(cntA + cntB), axis=1, kind="stable")  # [8, NW]
    slot_of = np.empty_like(perm)
    np.put_along_axis(slot_of, perm, np.tile(np.arange(NW), (NCORES, 1)), 1)
    capA = np.take_along_axis(cntA, perm, 1).max(0)  # [NW]
    capB = np.take_along_axis(cntB, perm, 1).max(0)
    A_sub_j = np.maximum(1, -(-capA // P)).astype(np.int64)
    B_sub_j = np.maximum(1, -(-capB // P)).astype(np.int64)
    W_SUB_j = A_sub_j + B_sub_j
    offE = np.zeros(NW + 1, np.int64)           # slot start, in edges
    np.cumsum(W_SUB_j * P, out=offE[1:])
    E_pad = int(offE[NW])
    S_total = E_pad // P

    # target slot for each edge in the padded per-core layout
    order = np.argsort(group, kind="stable")
    gs = group[order]
    starts = np.zeros(NG + 1, np.int64)
    np.cumsum(counts, out=starts[1:])
    within = np.arange(E, dtype=np.int64) - starts[gs]
    g_core = gs // (2 * NW)
    g_win = (gs // 2) % NW
    g_slot = slot_of[g_core, g_win]
    tgt = (g_core * E_pad + offE[g_slot]
           + (gs & 1) * A_sub_j[g_slot] * P + within)

    eid = np.full(NCORES * E_pad, -1, np.int64)
    eid[tgt] = order
    valid = eid >= 0
    eiv = eid[valid]

    ea_pad = np.zeros((NCORES * E_pad, H), bf16)
    ea_pad[valid] = ea.astype(bf16)[eiv]
    eaT = np.ascontiguousarray(
        ea_pad.reshape(NCORES, E_pad, H).transpose(0, 2, 1)
    )  # [8, 128, E_pad]

    kvidx = np.zeros(NCORES * E_pad, np.int64)  # pads gather row 0 (harmless)
    kvidx[valid] = src[eiv] - SPLIT * half[eiv]
    kvidx = kvidx.astype(np.int16).reshape(NCORES, E_pad)
    kvA = np.concatenate(
        [_wrap16(kvidx[:, offE[j]:offE[j] + A_sub_j[j] * P])
         for j in range(NW)], axis=2)            # [8, 128, sum(A_sub)*8]
    kvB = np.concatenate(
        [_wrap16(kvidx[:, offE[j] + A_sub_j[j] * P:offE[j + 1]])
         for j in range(NW)], axis=2)            # [8, 128, sum(B_sub)*8]
    kvA = np.ascontiguousarray(kvA)
    kvB = np.ascontiguousarray(kvB)

    rk = np.full(NCORES * E_pad, 300.0, np.float32)
    rk[valid] = rank[eiv]
    rankpt = np.ascontiguousarray(
        rk.reshape(NCORES, S_total, P).transpose(0, 2, 1)
    ).astype(bf16)  # [8, 128, S_total] bf16: [p, s] = rank of edge s*128+p
    rank_rows = np.ascontiguousarray(
        rk.reshape(NCORES, E_pad)
    ).astype(bf16)  # [8, E_pad]: row layout for partition_broadcast

    # feature-major node features
    x_bf = np.zeros((NODE_PAD, H), bf16)
    x_bf[:N] = x.astype(bf16)
    xT_bf = np.ascontiguousarray(x_bf.T)          # [128, NODE_PAD]
    x_ownT = np.zeros((NCORES, H, QROWS), bf16)   # feature-major, slot order
    x_own_f = np.zeros((NCORES, QROWS, H), np.float32)
    for c in range(NCORES):
        for j in range(NW):
            wid = int(perm[c, j])
            lo = c * NPC + wid * P
            n = min(P, NPC - wid * P)
            x_ownT[c, :, j * P:j * P + n] = x_bf[lo:lo + n].T
            x_own_f[c, j * P:j * P + n] = x[lo:lo + n]

    # LN folding: LN(v) @ W + b  ==  rsqrt(var) * (v @ Wc) + bc, with
    # Wc = (I - 11^T/128) diag(ln_w) W  and  bc = ln_b @ W + b.
    Cn = np.eye(H, dtype=f32) - np.full((H, H), 1.0 / H, f32)
    Wc_k = Cn @ (lsw[:, None] * Wk)
    Wc_v = Cn @ (lsw[:, None] * Wv)
    Wc_q = Cn @ (lsw[:, None] * Wq)
    Wc_ek = Cn @ (lew[:, None] * Wek)
    Wc_ev = Cn @ (lew[:, None] * Wev)
    mean_col = np.full((H, 1), 1.0 / H, f32)
    # [Wc_k | Wc_v | mean] -> [128, 257]
    Wc_kvo = np.concatenate([Wc_k, Wc_v, mean_col], 1).astype(bf16)
    Wc_ekvo = np.concatenate([Wc_ek, Wc_ev, mean_col], 1).astype(bf16)
    Wc_qo = np.concatenate([Wc_q, mean_col], 1).astype(bf16)      # [128, 129]
    ssq_col = mean_col.astype(bf16)                               # [128, 1]
    b_k = lsb @ Wk + bk + leb @ Wek + bek
    b_v = lsb @ Wv + bv + leb @ Wev + bev
    b_kv_rep = np.tile(np.concatenate([b_k, b_v])[None, :], (P, 1)).astype(bf16)
    b_q_rep = np.tile((lsb @ Wq + bq)[None, :], (P, 1)).astype(bf16)
    W1c = (Cn @ (lfw[:, None] * W1)).astype(bf16)             # [128, 512]
    b1_row = (lfb @ W1 + b1)[None, :].astype(bf16)            # [1, 512]
    W2p = np.ascontiguousarray(
        W2.reshape(4, P, H).transpose(1, 0, 2)
    ).astype(bf16)                                            # [128, 4, 128]
    b2_row = b2[None, :].astype(bf16)
    C_iota = np.tile(np.arange(P, dtype=f32)[None, :], (P, 1)).astype(bf16)
    p_iota = np.arange(P, dtype=f32)[:, None]                 # [128, 1] f32
    ident = np.eye(P, dtype=f32).astype(bf16)
    ones_row = np.ones((1, P), bf16)

    shared = dict(
        xT_bf=xT_bf, Wc_kvo=Wc_kvo, Wc_ekvo=Wc_ekvo, Wc_qo=Wc_qo,
        ssq_col=ssq_col, b_kv_rep=b_kv_rep, b_q_rep=b_q_rep,
        W1c=W1c, b1_row=b1_row, W2p=W2p, b2_row=b2_row,
        C_iota=C_iota, p_iota=p_iota, ident=ident, ones_row=ones_row,
    )
    in_maps = []
    for c in range(NCORES):
        m = dict(shared)
        m.update(
            eaT=eaT[c], kvA=kvA[c], kvB=kvB[c],
            rankpt=rankpt[c], rank_rows=rank_rows[c][None, :],
            x_ownT=x_ownT[c], x_own_f=x_own_f[c],
        )
        in_maps.append(m)

    cfg = dict(A_sub_j=tuple(int(v) for v in A_sub_j),
               B_sub_j=tuple(int(v) for v in B_sub_j),
               E_pad=E_pad, S_total=S_total)
    return cfg, in_maps, perm


def _build(cfg):
    A_sub_j = np.asarray(cfg["A_sub_j"], np.int64)
    B_sub_j = np.asarray(cfg["B_sub_j"], np.int64)
    W_SUB_j = A_sub_j + B_sub_j
    E_pad, S_total = cfg["E_pad"], cfg["S_total"]
    W_MAX = int(W_SUB_j.max())
    offE = np.zeros(NW + 1, np.int64)
    np.cumsum(W_SUB_j * P, out=offE[1:])
    offA16 = np.zeros(NW + 1, np.int64)       # kvA idx-table col offsets
    np.cumsum(A_sub_j * 8, out=offA16[1:])
    offB16 = np.zeros(NW + 1, np.int64)
    np.cumsum(B_sub_j * 8, out=offB16[1:])

    nc = bacc.Bacc("TRN2", target_bir_lowering=False, debug=False)

    # ---- I/O ----
    xT_bf_d = nc.dram_tensor("xT_bf", [P, NODE_PAD], BF, kind="ExternalInput")
    x_ownT_d = nc.dram_tensor("x_ownT", [P, QROWS], BF, kind="ExternalInput")
    x_own_f_d = nc.dram_tensor("x_own_f", [QROWS, H], F32, kind="ExternalInput")
    eaT_d = nc.dram_tensor("eaT", [P, E_pad], BF, kind="ExternalInput")
    kvA_d = nc.dram_tensor("kvA", [P, int(offA16[NW])], I16,
                           kind="ExternalInput")
    kvB_d = nc.dram_tensor("kvB", [P, int(offB16[NW])], I16,
                           kind="ExternalInput")
    rankpt_d = nc.dram_tensor("rankpt", [P, S_total], BF, kind="ExternalInput")
    rank_rows_d = nc.dram_tensor("rank_rows", [1, E_pad], BF,
                                 kind="ExternalInput")
    Wc_kvo_d = nc.dram_tensor("Wc_kvo", [P, 257], BF, kind="ExternalInput")
    Wc_ekvo_d = nc.dram_tensor("Wc_ekvo", [P, 257], BF, kind="ExternalInput")
    Wc_qo_d = nc.dram_tensor("Wc_qo", [P, 129], BF, kind="ExternalInput")
    ssq_col_d = nc.dram_tensor("ssq_col", [P, 1], BF, kind="ExternalInput")
    b_kv_d = nc.dram_tensor("b_kv_rep", [P, 256], BF, kind="ExternalInput")
    b_q_d = nc.dram_tensor("b_q_rep", [P, P], BF, kind="ExternalInput")
    W1c_d = nc.dram_tensor("W1c", [P, 4 * H], BF, kind="ExternalInput")
    b1_d = nc.dram_tensor("b1_row", [1, 4 * H], BF, kind="ExternalInput")
    W2p_d = nc.dram_tensor("W2p", [P, 4, H], BF, kind="ExternalInput")
    b2_d = nc.dram_tensor("b2_row", [1, H], BF, kind="ExternalInput")
    iota_d = nc.dram_tensor("C_iota", [P, P], BF, kind="ExternalInput")
    p_iota_d = nc.dram_tensor("p_iota", [P, 1], F32, kind="ExternalInput")
    ident_d = nc.dram_tensor("ident", [P, P], BF, kind="ExternalInput")
    ones_d = nc.dram_tensor("ones_row", [1, P], BF, kind="ExternalInput")
    out_d = nc.dram_tensor("out", [QROWS, H], F32, kind="ExternalOutput")

    with tile.TileContext(nc) as tc, ExitStack() as ctx:
        const = ctx.enter_context(tc.tile_pool(name="const", bufs=1))

        kvn_t = nc.dram_tensor("kvn_s", [NODE_PAD, 256], BF,
                               kind="ExternalOutput")
        qn_t = nc.dram_tensor("qn_s", [QROWS, H], BF, kind="ExternalOutput")

        # resident constants
        wckvo = const.tile([P, 257], BF)
        wcekvo = const.tile([P, 257], BF)
        wcqo = const.tile([P, 129], BF)
        ssqc = const.tile([P, 1], BF)
        bkv = const.tile([P, 256], BF)
        bqr = const.tile([P, P], BF)
        w1c = const.tile([P, 4 * H], BF)
        b1r = const.tile([1, 4 * H], BF)
        w2p = const.tile([P, 4, H], BF)
        b2r = const.tile([1, H], BF)
        iota = const.tile([P, P], BF)
        piota = const.tile([P, 1], F32)
        idn = const.tile([P, P], BF)
        onesr = const.tile([1, P], BF)
        rank_sb = const.tile([P, S_total], BF)
        kvA_sb = const.tile([P, int(offA16[NW])], I16)
        kvB_sb = const.tile([P, int(offB16[NW])], I16)
        eps_c = const.tile([P, 1], F32)
        tiny_c = const.tile([P, 1], F32)
        nc.vector.memset(eps_c[:], EPS)
        nc.vector.memset(tiny_c[:], 1e-16)
        for t, d in ((wckvo, Wc_kvo_d), (wcekvo, Wc_ekvo_d), (wcqo, Wc_qo_d),
                     (ssqc, ssq_col_d), (bkv, b_kv_d), (bqr, b_q_d),
                     (w1c, W1c_d), (b1r, b1_d), (w2p, W2p_d), (b2r, b2_d),
                     (iota, iota_d), (piota, p_iota_d), (idn, ident_d),
                     (onesr, ones_d), (rank_sb, rankpt_d),
                     (kvA_sb, kvA_d), (kvB_sb, kvB_d)):
            nc.sync.dma_start(out=t[:], in_=d[:])

        # ---------------- node phase ----------------
        # kvn = rs * (x @ Wc_kv) + b, processed from feature-major xT.
        # mean comes from col 256 of the proj matmul, ssq from a second
        # matmul with squared operand; rs = rsqrt(E[x^2] - mu^2 + eps).
        def project_nodes(xT_dram, nrows, wc, wid, brep, dst_dram, tag):
            nsub = nrows // P
            with ExitStack() as c2:
                sb = c2.enter_context(tc.tile_pool(name=f"np_{tag}", bufs=3))
                ps = c2.enter_context(
                    tc.tile_pool(name=f"npp_{tag}", bufs=4, space="PSUM"))
                for g in range(0, nsub, MACRO_N):
                    gn = min(MACRO_N, nsub - g)
                    slab = sb.tile([P, MACRO_N, P], BF, tag="slab")
                    nc.sync.dma_start(
                        out=slab[:, 0:gn, :],
                        in_=xT_dram[:, g * P:(g + gn) * P].rearrange(
                            "p (t c) -> p t c", c=P))
                    sq = sb.tile([P, MACRO_N, P], BF, tag="sq")
                    nc.vector.tensor_mul(out=sq[:, 0:gn, :],
                                         in0=slab[:, 0:gn, :],
                                         in1=slab[:, 0:gn, :])
                    stage = sb.tile([P, MACRO_N, wid], BF, tag="stage")
                    stats = sb.tile([P, MACRO_N, 2], F32, tag="stats")
                    for j in range(gn):
                        pp = ps.tile([P, wid + 2], F32, tag="pp")
                        nc.tensor.matmul(out=pp[:, 0:wid + 1],
                                         lhsT=slab[:, j, :], rhs=wc[:],
                                         start=True, stop=True)
                        nc.tensor.matmul(out=pp[:, wid + 1:wid + 2],
                                         lhsT=sq[:, j, :], rhs=ssqc[:],
                                         start=True, stop=True,
                                         skip_group_check=True)
                        # var = E[x^2] - mu^2 ; rs = rsqrt(var + eps)
                        nc.scalar.activation(out=stats[:, j, 0:1],
                                             in_=pp[:, wid:wid + 1],
                                             func=AF.Square)
                        nc.vector.tensor_sub(out=stats[:, j, 1:2],
                                             in0=pp[:, wid + 1:wid + 2],
                                             in1=stats[:, j, 0:1])
                        sd = sb.tile([P, 1], F32, tag="sd")
                        nc.scalar.activation(out=sd[:], in_=stats[:, j, 1:2],
                                             func=AF.Sqrt, bias=eps_c[:])
                        rs = sb.tile([P, 1], F32, tag="rs")
                        nc.vector.reciprocal(out=rs[:], in_=sd[:])
                        nc.vector.scalar_tensor_tensor(
                            out=stage[:, j, :], in0=pp[:, 0:wid],
                            scalar=rs[:], in1=brep[:, 0:wid],
                            op0=ALU.mult, op1=ALU.add)
                    rows = gn * P
                    nc.sync.dma_start(
                        out=dst_dram[g * P:g * P + rows, :].rearrange(
                            "(t p) c -> p t c", p=P),
                        in_=stage[:, 0:gn, :])

        project_nodes(xT_bf_d, NODE_PAD, wckvo, 256, bkv, kvn_t, "kv")
        project_nodes(x_ownT_d, QROWS, wcqo, 128, bqr, qn_t, "q")

        # ---------------- edge phase ----------------
        with ExitStack() as c2:
            sbw = c2.enter_context(tc.tile_pool(name="win", bufs=2))
            sbe = c2.enter_context(tc.tile_pool(name="edge", bufs=3))
            ps_pp = c2.enter_context(
                tc.tile_pool(name="ppp", bufs=5, space="PSUM"))
            ps_qt = c2.enter_context(
                tc.tile_pool(name="pqt", bufs=1, space="PSUM"))
            ps_acc = c2.enter_context(
                tc.tile_pool(name="pacc", bufs=2, space="PSUM"))
            sbf = c2.enter_context(tc.tile_pool(name="ffn", bufs=2))

            for w in range(NW):
                A_sub = int(A_sub_j[w])
                B_sub = int(B_sub_j[w])
                W_SUB = A_sub + B_sub
                AE, BE = A_sub * P, B_sub * P
                e0 = int(offE[w])
                NMACRO = _ceil(W_SUB, MACRO)
                # window-level loads
                ea_slab = sbw.tile([P, W_SUB, P], BF, tag="eas")
                nc.sync.dma_start(out=ea_slab[:],
                                  in_=eaT_d[:, e0:e0 + W_SUB * P].rearrange(
                                      "p (t c) -> p t c", c=P))
                q_win = sbw.tile([P, P], BF, tag="qw")
                nc.sync.dma_start(out=q_win[:],
                                  in_=qn_t[w * P:(w + 1) * P, :])
                rr1 = sbw.tile([1, W_SUB * P], BF, tag="rr1")
                nc.sync.dma_start(out=rr1[:],
                                  in_=rank_rows_d[:, e0:e0 + W_SUB * P])
                rb = sbw.tile([P, W_SUB, P], BF, tag="rb")
                nc.gpsimd.partition_broadcast(rb[:], rr1[:])
                kv_g = sbw.tile([P, W_SUB, 256], BF, tag="kvg")
                for j in range(_ceil(AE, GC)):
                    n_i = min(GC, AE - j * GC)
                    c16 = int(offA16[w]) + j * (GC // 16)
                    nc.gpsimd.dma_gather(
                        kv_g[:, j * (GC // P):j * (GC // P) + n_i // P, :],
                        kvn_t[0:SPLIT, :],
                        kvA_sb[:, c16:c16 + n_i // 16],
                        n_i, n_i, 256)
                for j in range(_ceil(BE, GC)):
                    n_i = min(GC, BE - j * GC)
                    c16 = int(offB16[w]) + j * (GC // 16)
                    nc.gpsimd.dma_gather(
                        kv_g[:, A_sub + j * (GC // P):
                             A_sub + j * (GC // P) + n_i // P, :],
                        kvn_t[SPLIT:NODE_PAD, :],
                        kvB_sb[:, c16:c16 + n_i // 16],
                        n_i, n_i, 256)

                agg = ps_acc.tile([P, 136], F32, tag="acc")

                for m in range(NMACRO):
                    mn = min(MACRO, W_SUB - m * MACRO)
                    gs0 = e0 // P + m * MACRO
                    ea4 = ea_slab[:, m * MACRO:m * MACRO + mn, :]
                    sq4 = sbe.tile([P, mn, P], BF, tag="sq4")
                    nc.vector.tensor_mul(out=sq4[:], in0=ea4, in1=ea4)
                    pps = []
                    m2_4 = sbe.tile([P, mn], F32, tag="m2_4")
                    var4 = sbe.tile([P, mn], F32, tag="var4")
                    sd4 = sbe.tile([P, mn], F32, tag="sd4")
                    for s in range(mn):
                        pp = ps_pp.tile([P, 258], F32, tag="pp")
                        pps.append(pp)
                        nc.tensor.matmul(out=pp[:, 0:257],
                                         lhsT=ea4[:, s, :], rhs=wcekvo[:],
                                         start=True, stop=True,
                                         skip_group_check=True)
                        nc.tensor.matmul(out=pp[:, 257:258],
                                         lhsT=sq4[:, s, :], rhs=ssqc[:],
                                         start=True, stop=True,
                                         skip_group_check=True)
                        # var = E[x^2] - mu^2 ; sd = sqrt(var + eps)
                        nc.scalar.activation(out=m2_4[:, s:s + 1],
                                             in_=pp[:, 256:257],
                                             func=AF.Square)
                        nc.vector.tensor_sub(out=var4[:, s:s + 1],
                                             in0=pp[:, 257:258],
                                             in1=m2_4[:, s:s + 1])
                        nc.scalar.activation(out=sd4[:, s:s + 1],
                                             in_=var4[:, s:s + 1],
                                             func=AF.Sqrt, bias=eps_c[:])
                    rs4 = sbe.tile([P, mn], F32, tag="rs4")
                    nc.vector.reciprocal(out=rs4[:], in_=sd4[:])
                    # kvf = kv_g + rs * ekv   (fused PSUM->SBUF)
                    kvf4 = sbe.tile([P, mn, 256], BF, tag="kvf4")
                    for s in range(mn):
                        nc.vector.scalar_tensor_tensor(
                            out=kvf4[:, s, :], in0=pps[s][:, 0:256],
                            scalar=rs4[:, s:s + 1],
                            in1=kv_g[:, m * MACRO + s, :],
                            op0=ALU.mult, op1=ALU.add)
                    # S_T[n, e] = (n == rank(e)) ; q = S_T^T @ Q_win
                    st_4 = sbe.tile([P, mn, P], BF, tag="stq4")
                    nc.vector.tensor_scalar(
                        out=st_4[:], in0=rb[:, m * MACRO:m * MACRO + mn, :],
                        scalar1=piota[:], scalar2=None, op0=ALU.is_equal)
                    qt4 = ps_qt.tile([P, mn, P], F32, tag="qt")
                    for s in range(mn):
                        nc.tensor.matmul(out=qt4[:, s, :],
                                         lhsT=st_4[:, s, :], rhs=q_win[:],
                                         start=True, stop=True,
                                         skip_group_check=True)
                    # logits and softmax numerators
                    qk4 = sbe.tile([P, mn, P], BF, tag="qk4")
                    nc.vector.tensor_mul(out=qk4[:], in0=qt4[:],
                                         in1=kvf4[:, :, 0:P])
                    l4 = sbe.tile([P, mn, NH], F32, tag="l4")
                    nc.vector.tensor_reduce(
                        out=l4[:],
                        in_=qk4[:].rearrange("p m (h d) -> p m h d", d=HD),
                        axis=mybir.AxisListType.X, op=ALU.add)
                    U4 = sbe.tile([P, mn, 136], BF, tag="U4")
                    nc.scalar.activation(out=U4[:, :, P:136], in_=l4[:],
                                         func=AF.Exp, scale=0.25)
                    nc.vector.tensor_mul(
                        out=U4[:, :, 0:P].rearrange("p m (h d) -> p m h d", d=HD),
                        in0=kvf4[:, :, P:256].rearrange("p m (h d) -> p m h d", d=HD),
                        in1=U4[:, :, P:136].unsqueeze(3).broadcast_to(
                            [P, mn, NH, HD]))
                    # S[e, n] = (rank(e) == n) ; agg += S^T @ U
                    s4 = sbe.tile([P, mn, P], BF, tag="s4")
                    nc.vector.tensor_tensor(
                        out=s4[:],
                        in0=iota[:].unsqueeze(1).broadcast_to([P, mn, P]),
                        in1=rank_sb[:, gs0:gs0 + mn].unsqueeze(2)
                            .broadcast_to([P, mn, P]),
                        op=ALU.is_equal)
                    for s in range(mn):
                        nc.tensor.matmul(out=agg[:], lhsT=s4[:, s, :],
                                         rhs=U4[:, s, :],
                                         start=(m == 0 and s == 0),
                                         stop=(m == NMACRO - 1 and s == mn - 1))

                # ---- finalize + FFN for this window ----
                den = sbf.tile([P, NH], F32, tag="den")
                nc.scalar.activation(out=den[:], in_=agg[:, P:136],
                                     func=AF.Copy, bias=1e-16)
                rden = sbf.tile([P, NH], F32, tag="rden")
                nc.vector.reciprocal(out=rden[:], in_=den[:])
                xw = sbf.tile([P, H], F32, tag="xw")
                nc.sync.dma_start(out=xw[:],
                                  in_=x_own_f_d[w * P:(w + 1) * P, :])
                aggn = sbf.tile([P, H], F32, tag="aggn")
                nc.vector.tensor_mul(
                    out=aggn[:].rearrange("p (h d) -> p h d", d=HD),
                    in0=agg[:, 0:H].rearrange("p (h d) -> p h d", d=HD),
                    in1=rden[:].unsqueeze(2).broadcast_to([P, NH, HD]))
                xd = sbf.tile([P, H], F32, tag="xd")
                nc.vector.tensor_add(out=xd[:], in0=xw[:], in1=aggn[:])

                st6f = sbf.tile([P, 6], F32, tag="st6f")
                mvf = sbf.tile([P, 2], F32, tag="mvf")
                nc.vector.bn_stats(out=st6f[:], in_=xd[:])
                nc.vector.bn_aggr(out=mvf[:], in_=st6f[:])
                sdf = sbf.tile([P, 1], F32, tag="sdf")
                nc.scalar.activation(out=sdf[:], in_=mvf[:, 1:2],
                                     func=AF.Sqrt, bias=eps_c[:])
                rsf = sbf.tile([P, 1], F32, tag="rsf")
                nc.vector.reciprocal(out=rsf[:], in_=sdf[:])
                hp = sbf.tile([P, H], BF, tag="hp")
                nc.vector.tensor_scalar_mul(out=hp[:], in0=xd[:], scalar1=rsf[:])
                hT_ps = ps_acc.tile([P, P], BF, tag="acc")
                nc.tensor.transpose(out=hT_ps[:], in_=hp[:], identity=idn[:])
                hT = sbf.tile([P, P], BF, tag="hT")
                nc.vector.tensor_copy(out=hT[:], in_=hT_ps[:])
                h1 = ps_acc.tile([P, 4 * H], F32, tag="acc")
                nc.tensor.matmul(out=h1[:], lhsT=hT[:], rhs=w1c[:],
                                 start=True, stop=False)
                nc.tensor.matmul(out=h1[:], lhsT=onesr[:], rhs=b1r[:],
                                 start=False, stop=True)
                r = sbf.tile([P, 4 * H], BF, tag="r")
                nc.scalar.activation(out=r[:], in_=h1[:], func=AF.Relu)
                rT_ps = ps_acc.tile([P, 4 * H], BF, tag="acc")
                for k in range(4):
                    nc.tensor.transpose(out=rT_ps[:, k * P:(k + 1) * P],
                                        in_=r[:, k * P:(k + 1) * P],
                                        identity=idn[:])
                rT = sbf.tile([P, 4 * H], BF, tag="rT")
                nc.vector.tensor_copy(out=rT[:], in_=rT_ps[:])
                op = ps_acc.tile([P, H], F32, tag="acc")
                for k in range(4):
                    nc.tensor.matmul(out=op[:], lhsT=rT[:, k * P:(k + 1) * P],
                                     rhs=w2p[:, k, :], start=(k == 0),
                                     stop=False)
                nc.tensor.matmul(out=op[:], lhsT=onesr[:], rhs=b2r[:],
                                 start=False, stop=True)
                ob = sbf.tile([P, H], F32, tag="ob")
                nc.vector.tensor_add(out=ob[:], in0=xd[:], in1=op[:])
                nc.sync.dma_start(out=out_d[w * P:(w + 1) * P, :], in_=ob[:])

    nc.compile()
    return nc


_CACHE = {}


def _get_program(cfg):
    key = tuple(sorted(cfg.items()))
    if key not in _CACHE:
        _CACHE[key] = _build(cfg)
    return _CACHE[key]


def kernel(_collect_results=None, **inputs):
    cfg, in_maps, perm = _prep(inputs)
    nc = _get_program(cfg)
    res = run_bass_kernel_spmd(
        nc, in_maps, core_ids=list(range(NCORES)),
        trace=bool(os.environ.get("GNN_TRACE", "")))
    if _collect_results is not None:
        _collect_results.append(res)
    out = np.empty((N, H), np.float32)
    for c in range(NCORES):
        oc = res.results[c]["out"]
        for j in range(NW):
            wid = int(perm[c, j])
            n = min(P, NPC - wid * P)
            out[c * NPC + wid * P:c * NPC + wid * P + n] = oc[j * P:j * P + n]
    return out
